# revision 1
# baseline (speedup 1.0000x reference)
"""Causal self-attention on 8 trn2 NeuronCores — v2.

Sharding: DP4 (batch) x TP2 (head groups of 8). Core c -> batch c//2,
head group c%2. Each core computes qkv^T for its 512 channels, causal
attention for its 8 heads over all T=2048 queries, and a partial
projection y_partial = O_g @ W_proj[rows_g] (+ b_proj on group 0).
Host sums the two partials per batch and transposes (kernel emits y^T).

v2 structure (HW-microbenchmark calibrated):
- Scores S^T = K Q^T per head run as row-group-packed matmul pairs
  (tile_position (0,0)/(64,0), K=64 each) -> ~158 ns per logical MM.
- One exp over [128,1024] PSUM (both heads of a pair, one k-block)
  -> ~0.99us per i-block; Act engine does nothing else.
- Causal masking via gpsimd affine_select (no mask tensors).
- PV (attention @ V) in bf16 with the ones-column row-sum trick
  (M=65), two head-chains interleaved across PSUM banks -> ~189 ns/MM.
- qkv/proj accumulation chains interleaved pairwise across the two
  banks of a shared [128,1024] PSUM tile; PSUM->SBUF evacuation with
  bias on DVE/Pool (tensor_scalar), not Act.
- Phases pipelined: A(s) qkv slice -> B(j=s) attention -> C(s) proj,
  all under one Tile dependency graph.
"""
import sys

sys.path.insert(0, "/opt/trn_rl_repo")

import numpy as np

import concourse.bass as bass
import concourse.tile as tile
from concourse import bacc, mybir

f32 = mybir.dt.float32
f32r = mybir.dt.float32r
bf16 = mybir.dt.bfloat16
AFT = mybir.ActivationFunctionType
ALU = mybir.AluOpType

N_CORES = 8
B, T, C = 4, 2048, 1024
H, HD = 16, 64            # total heads, head dim
HPC = 8                   # heads per core
CPC = 512                 # channels per core (q, k or v)
NT = T // 128             # 16 t-tiles of 128
NS = T // 512             # 4 t-slices of 512
NC_T = C // 128           # 8 C-tiles (contraction)
SCALE = 1.0 / np.sqrt(HD)

# bisection flags
PACK_SCORES = True        # row-group-packed score matmul pairs
FAST_RECIP = False         # reciprocal_approx_fast vs reciprocal


def build_nc(repeat: int = 1):
    nc = bacc.Bacc("TRN2", target_bir_lowering=False, debug=False,
                   num_devices=N_CORES)

    xb_d = nc.dram_tensor("xb", [C, T], f32, kind="ExternalInput")
    wqkv_d = nc.dram_tensor("wqkv", [C, 3 * CPC], f32, kind="ExternalInput")
    bqkv_d = nc.dram_tensor("bqkv", [128, 12], f32, kind="ExternalInput")
    wp_d = nc.dram_tensor("wp", [CPC, C], f32, kind="ExternalInput")
    bp_d = nc.dram_tensor("bp", [128, 8], f32, kind="ExternalInput")
    yt_d = nc.dram_tensor("yT", [C, T], f32, kind="ExternalOutput")

    with tile.TileContext(nc) as tc:
        def body(_=None):
            _build_body(nc, tc, xb_d, wqkv_d, bqkv_d, wp_d, bp_d, yt_d)
        if repeat == 1:
            body()
        else:
            with tc.For_i(0, repeat, 1):
                body()
    nc.compile()
    return nc


def _build_body(nc, tc, xb_d, wqkv_d, bqkv_d, wp_d, bp_d, yt_d):
    pers_cm = tc.tile_pool(name="pers", bufs=1)
    pers = pers_cm.__enter__()

    bqkv = pers.tile([128, 12], f32, name="bqkv")
    nc.sync.dma_start(bqkv[:], bqkv_d.ap())
    bp = pers.tile([128, 8], f32, name="bp")
    nc.sync.dma_start(bp[:], bp_d.ap())

    # weights (DVE queue so xb DMAs on sync aren't delayed)
    wqkv = [pers.tile([128, 3 * CPC], f32r, name=f"wqkv{ci}")
            for ci in range(NC_T)]
    for ci in range(NC_T):
        nc.scalar.dma_start(
            wqkv[ci][:],
            wqkv_d.ap()[128 * ci:128 * ci + 128, :].bitcast(f32r))
    wp = [pers.tile([128, C], f32r, name=f"wp{i}") for i in range(4)]
    for ci in range(4):
        nc.scalar.dma_start(
            wp[ci][:], wp_d.ap()[128 * ci:128 * ci + 128, :].bitcast(f32r))

    # qkv^T results
    qt = [pers.tile([128, T], f32r, name=f"qt{i}") for i in range(4)]
    kt = [pers.tile([128, T], f32r, name=f"kt{i}") for i in range(4)]
    # V natural + ones col per head (bf16)
    vaug = [pers.tile([128, 8 * 65], bf16, name=f"vaug{i}") for i in range(NT)]
    for i in range(NT):
        nc.gpsimd.memset(vaug[i][:], 1.0)

    with tc.tile_pool(name="xt", bufs=10) as xt_pool, \
         tc.tile_pool(name="pt", bufs=4) as pt_pool, \
         tc.tile_pool(name="oc", bufs=4) as oc_pool, \
         tc.tile_pool(name="rl", bufs=2) as rl_pool, \
         tc.tile_pool(name="rlb", bufs=2) as rlb_pool, \
         tc.tile_pool(name="otp", bufs=6) as ot_pool, \
         tc.tile_pool(name="ytp", bufs=2) as yt_pool, \
         tc.tile_pool(name="pac", bufs=2, space="PSUM") as pac_pool, \
         tc.tile_pool(name="pst", bufs=2, space="PSUM") as pst_pool, \
         tc.tile_pool(name="ots", bufs=2, space="PSUM") as ots_pool:

        pools = dict(xt=xt_pool, pt=pt_pool, oc=oc_pool, rl=rl_pool,
                     rlb=rlb_pool, ot=ot_pool, yt=yt_pool, pst=pst_pool,
                     ots=ots_pool)
        ot_tiles = {}

        for s in range(NS):
            xts = _xt_dmas(nc, s, xb_d, xt_pool, 0, 8)
            _phase_a_slice(nc, s, xb_d, wqkv, bqkv, qt, kt, vaug,
                           xts, pac_pool)
            _phase_b_qtile(nc, s, qt, kt, vaug, bqkv, pools, ot_tiles)
            _phase_c_slice(nc, s, wp, bp, yt_d, ot_tiles,
                           yt_pool, pac_pool)

    pers_cm.__exit__(None, None, None)


def _xt_dmas(nc, s, xb_d, xt_pool, ci0, ci1):
    xts = []
    for ci in range(ci0, ci1):
        xtt = xt_pool.tile([128, 512], f32r, name="xt")
        nc.sync.dma_start(
            xtt[:],
            xb_d.ap()[128 * ci:128 * ci + 128,
                      512 * s:512 * s + 512].bitcast(f32r))
        xts.append(xtt)
    return xts


def _phase_a_slice(nc, s, xb_d, wqkv, bqkv, qt, kt, vaug, xts, pac_pool):
    """qkv^T for t-slice s (512 timesteps); xts pre-DMA'd."""

    # Q (g=0..3) / K (g=4..7): out[c_out 128, t 512], two chains
    # interleaved across the pool's two banks
    for gp in range(4):
        g0, g1 = 2 * gp, 2 * gp + 1
        ps0 = pac_pool.tile([128, 512], f32, name="pac")
        ps1 = pac_pool.tile([128, 512], f32, name="pac")
        for ci in range(NC_T):
            nc.tensor.matmul(ps0[:],
                             wqkv[ci][:, 128 * g0:128 * g0 + 128], xts[ci][:],
                             start=(ci == 0), stop=(ci == NC_T - 1))
            nc.tensor.matmul(ps1[:],
                             wqkv[ci][:, 128 * g1:128 * g1 + 128], xts[ci][:],
                             start=(ci == 0), stop=(ci == NC_T - 1))
        for src, g in ((ps0, g0), (ps1, g1)):
            if g < 4:
                dst = qt[g][:, 512 * s:512 * s + 512]
                # bias pre-scaled on host: (ps*SCALE) + bias
                nc.vector.tensor_scalar(dst, src[:], SCALE,
                                        bqkv[:, g:g + 1],
                                        ALU.mult, ALU.add)
            else:
                dst = kt[g - 4][:, 512 * s:512 * s + 512]
                nc.vector.tensor_scalar_add(dst, src[:], bqkv[:, g:g + 1])

    # V: out[t 128, c_v 512], two t-tile chains interleaved
    for tp in range(2):
        ps0 = pac_pool.tile([128, 512], f32, name="pac")
        ps1 = pac_pool.tile([128, 512], f32, name="pac")
        for ci in range(NC_T):
            for ps, tt in ((ps0, 2 * tp), (ps1, 2 * tp + 1)):
                nc.tensor.matmul(
                    ps[:],
                    xts[ci][:, 128 * tt:128 * tt + 128],
                    wqkv[ci][:, 1024:1536],
                    start=(ci == 0), stop=(ci == NC_T - 1))
        for ps, half in ((ps0, 0), (ps1, 1)):
            ti = 4 * s + 2 * tp + half
            dst = vaug[ti][:].rearrange("p (h w) -> p h w", w=65)[:, :, 0:64]
            nc.vector.tensor_copy(
                dst, ps[:].rearrange("p (h w) -> p h w", w=64))


def _phase_b_qtile(nc, j, qt, kt, vaug, bqkv, pools, ot_tiles):
    """Attention for q-tile j (512 queries), all 4 head pairs."""
    pst_pool, ots_pool = pools["pst"], pools["ots"]
    pt_pool, oc_pool = pools["pt"], pools["oc"]
    for hp in range(4):
        hA, hB = 2 * hp, 2 * hp + 1
        oa = ots_pool.tile([65, 512], f32, name="ots")
        ob = ots_pool.tile([65, 512], f32, name="ots")
        i_max = 4 * j + 3
        for i in range(i_max + 1):
            o = i - 4 * j
            # valid query columns start at 128*o within this 512-q block;
            # keep the score matmul at N>=256 (f32r full-rate floor)
            q0v = max(0, 128 * o)
            q0 = min(q0v, 256)
            nv = 512 - q0v
            ps = pst_pool.tile([128, 1024], f32, name="pst")
            # packed score pair: head A on rows 0-63 (T0), B on 64-127 (T8)
            tp_kw = ({"tile_position": (0, 0)}, {"tile_position": (64, 0)}) \
                if PACK_SCORES else ({}, {})
            nc.tensor.matmul(ps[:, q0:512],
                             kt[hp][0:64, 128 * i:128 * i + 128],
                             qt[hp][0:64, 512 * j + q0:512 * j + 512],
                             start=True, stop=True, **tp_kw[0])
            nc.tensor.matmul(ps[:, 512 + q0:1024],
                             kt[hp][64:128, 128 * i:128 * i + 128],
                             qt[hp][64:128, 512 * j + q0:512 * j + 512],
                             start=True, stop=True, **tp_kw[1])
            ptile = pt_pool.tile([128, 1024], bf16, name="pt")
            if q0v == 0:
                nc.scalar.activation(ptile[:], ps[:], AFT.Exp)
            else:
                # two contiguous exps (strided 3D activation is slow)
                nc.scalar.activation(ptile[:, q0v:512], ps[:, q0v:512],
                                     AFT.Exp)
                nc.scalar.activation(ptile[:, 512 + q0v:1024],
                                     ps[:, 512 + q0v:1024], AFT.Exp)
            if o >= 0:
                # zero the strict upper triangle of the diagonal 128x128
                # sub-block: keep where (q - k) >= 0, q local to the block
                tri = ptile[:].rearrange("p (g q) -> p g q",
                                         q=512)[:, :, q0v:q0v + 128]
                nc.gpsimd.affine_select(
                    out=tri, in_=tri, compare_op=ALU.is_ge, fill=0.0,
                    base=0, channel_multiplier=-1,
                    pattern=[[0, 2], [1, 128]])
            nc.tensor.matmul(oa[:, q0v:512],
                             vaug[i][:, 65 * hA:65 * hA + 65],
                             ptile[:, q0v:512],
                             start=(i == 0), stop=(i == i_max))
            nc.tensor.matmul(ob[:, q0v:512],
                             vaug[i][:, 65 * hB:65 * hB + 65],
                             ptile[:, 512 + q0v:1024],
                             start=(i == 0), stop=(i == i_max))
        # evacuate PSUM accumulators to SBUF fast, then normalize there
        ot = pools["ot"].tile([128, 512], f32r, name="ot")
        for hl, po in ((0, oa), (1, ob)):
            oc = oc_pool.tile([65, 512], f32, name="oc")
            nc.vector.tensor_copy(oc[:], po[:])
            rl = pools["rl"].tile([1, 512], f32, name="rl")
            nc.vector.reciprocal(rl[:], oc[64:65, :])
            rlb = pools["rlb"].tile([64, 512], f32, name="rlb")
            nc.gpsimd.partition_broadcast(rlb[:], rl[:])
            dst = ot[64 * hl:64 * hl + 64, :]
            nc.vector.tensor_mul(dst, oc[0:64, :], rlb[:])
            nc.vector.tensor_scalar_add(
                dst, dst, bqkv[64 * hl:64 * hl + 64, 8 + hp:9 + hp])
        ot_tiles[(hp, j)] = ot


def _phase_c_slice(nc, s, wp, bp, yt_d, ot_tiles, yt_pool, pac_pool):
    """Projection for t-slice s: yT[:, cols s] = sum_ci wp[ci].T @ O^T."""
    for gp in range(4):
        g0, g1 = 2 * gp, 2 * gp + 1
        ps0 = pac_pool.tile([128, 512], f32, name="pac")
        ps1 = pac_pool.tile([128, 512], f32, name="pac")
        for ci in range(4):
            nc.tensor.matmul(ps0[:],
                             wp[ci][:, 128 * g0:128 * g0 + 128],
                             ot_tiles[(ci, s)][:],
                             start=(ci == 0), stop=(ci == 3))
            nc.tensor.matmul(ps1[:],
                             wp[ci][:, 128 * g1:128 * g1 + 128],
                             ot_tiles[(ci, s)][:],
                             start=(ci == 0), stop=(ci == 3))
        for ps, g in ((ps0, g0), (ps1, g1)):
            yt = yt_pool.tile([128, 512], f32, name="yt")
            nc.vector.tensor_scalar_add(
                yt[:], ps[:], bp[:, g:g + 1])
            nc.sync.dma_start(
                yt_d.ap()[128 * g:128 * g + 128, 512 * s:512 * s + 512],
                yt[:])


def make_inputs(x, W_attn, b_attn, W_proj, b_proj):
    """Host-side sharding: per-core input dicts."""
    x = np.asarray(x, np.float32)
    W_attn = np.asarray(W_attn, np.float32)
    b_attn = np.asarray(b_attn, np.float32)
    W_proj = np.asarray(W_proj, np.float32)
    b_proj = np.asarray(b_proj, np.float32)

    in_maps = []
    for core in range(N_CORES):
        b, g = divmod(core, 2)
        cols = np.concatenate([
            np.arange(CPC * g, CPC * g + CPC),
            C + np.arange(CPC * g, CPC * g + CPC),
            2 * C + np.arange(CPC * g, CPC * g + CPC)])
        wqkv = np.ascontiguousarray(W_attn[:, cols])
        bq = b_attn[cols].copy()                      # [1536]
        bq[:CPC] *= SCALE                             # fold q-scale into bias
        bqkv = np.ascontiguousarray(bq.reshape(12, 128).T)
        wp = np.ascontiguousarray(W_proj[CPC * g:CPC * g + CPC, :])
        bpv = (b_proj if g == 0 else np.zeros(C, np.float32))
        bpv = np.ascontiguousarray(bpv.reshape(8, 128).T)
        in_maps.append({
            "xb": np.ascontiguousarray(x[b].T),
            "wqkv": wqkv,
            "bqkv": bqkv,
            "wp": wp,
            "bp": bpv,
        })
    return in_maps


def unshard(results):
    """Combine per-core yT partials into [B, T, C] output."""
    out = np.empty((B, T, C), np.float32)
    for b in range(B):
        yt = results[2 * b]["yT"] + results[2 * b + 1]["yT"]
        out[b] = yt.T
    return out


_nc_cache = {}


def kernel(x, W_attn, b_attn, W_proj, b_proj):
    from concourse.bass_utils import run_bass_kernel_spmd
    if "nc" not in _nc_cache:
        _nc_cache["nc"] = build_nc(repeat=1)
    nc = _nc_cache["nc"]
    in_maps = make_inputs(x, W_attn, b_attn, W_proj, b_proj)
    res = run_bass_kernel_spmd(nc, in_maps, core_ids=list(range(N_CORES)),
                               trace=False)
    return unshard(res.results)

